# revision 5
# baseline (speedup 1.0000x reference)
"""Trainium2 Bass kernel for nn_CrossLevel (gnn_message_passing).

Reference semantics (see the problem's reference.py):

    AR_pairs = concat(output[H_edge_index[0]], Line_output[H_edge_index[1]], axis=1)
    AR_coff  = sigmoid(AR_pairs @ W.T + b).squeeze()          # in (0, 1), finite
    A        = zeros((H.shape[0], H.shape[1]))                # fresh zeros — AR_coff
                                                              # is never written into A
    out      = A @ Line_output + 0.0 * AR_coff.sum()

Exact-math analysis of that graph:

  * ``A`` is a fresh zeros matrix, so ``A @ Line_output`` is exactly +0.0
    everywhere (Line_output is finite).
  * ``sigmoid`` is bounded in (0, 1), so ``AR_coff.sum()`` over E edges is a
    finite positive float; ``0.0 * finite`` is exactly +0.0 in IEEE754.

Therefore the output is EXACTLY ``zeros((H.shape[0], Line_output.shape[1]),
float32)`` for every possible input: the gather+MLP stage is dead code (its
result is annihilated by the ``0.0 *`` factor — the original module computed
the edge coefficients but never scattered them into ``A``). The optimal
kernel eliminates the dead code and materializes that zeros tensor on the
device as fast as possible, taking the required HBM traffic from ~1 GB of
edge gathers down to the 10.24 MB output write itself.

Device strategy (8 NeuronCores): shard the output rows across the 8 cores
(2500 rows x 128 ch = 1.28 MB each). Per core, a single HWDGE DMA on the
sync (SP) ring writes the shard: the source is a tiny 10 KB zeros vector in
DRAM (an ExternalInput, uploaded before the timed NEFF execution) read
through a stride-0 broadcast access pattern, so no SBUF staging or memset
sits on the critical path. The transfer is bandwidth-bound (~1.28 MB at
~360 GB/s ~= 3.6 us) plus the fixed HWDGE/semaphore latency; the cost model
puts one core at ~7.1 us. The DMA is semaphore-synced and the program ends
only after the completion semaphore fires.
"""

import os

import numpy as np

N_CORES = 8
_ZLEN = 2500  # zeros-source length; 2500 f32 = 10 KB descriptors (>=4 KB
              # per descriptor saturates the DMA bus width; 128 descriptors
              # spread 8-per-engine across the 16 SDMA engines)


def _build_zero_writer(flat_elems: int):
    """Bass program: write ``flat_elems`` float32 zeros to the ``out`` DRAM
    tensor with one broadcast-source DMA. ``flat_elems`` must be a multiple
    of 128 * _ZLEN."""
    import concourse.bass as bass
    import concourse.mybir as mybir

    assert flat_elems % (128 * _ZLEN) == 0, flat_elems
    rep = flat_elems // (128 * _ZLEN)

    nc = bass.Bass()
    z_t = nc.declare_dram_parameter("z", [_ZLEN], mybir.dt.float32,
                                    isOutput=False)
    out_t = nc.declare_dram_parameter("out", [flat_elems], mybir.dt.float32,
                                      isOutput=True)
    out_ap = out_t[:].rearrange("(p r f) -> p r f", p=128, r=rep)
    src = z_t[0:_ZLEN].unsqueeze(0).unsqueeze(0).broadcast_to((128, rep, _ZLEN))

    with (nc.semaphore() as dma_sem, nc.Block() as block):
        @block.sync
        def _(sync):
            sync.dma_start(out=out_ap, in_=src).then_inc(dma_sem, 16)
            sync.wait_ge(dma_sem, 16)

    return nc


def _run_spmd(nc, in_maps, core_ids):
    """run_bass_kernel_spmd with a guard for containers where BASS_TRACE is
    set but the axon NTFF profiling hook module is absent (the trace path
    would raise ModuleNotFoundError before running anything)."""
    from concourse.bass_utils import run_bass_kernel_spmd

    try:
        return run_bass_kernel_spmd(nc, in_maps, core_ids=core_ids)
    except ModuleNotFoundError:
        os.environ["BASS_NEVER_TRACE"] = "1"
        return run_bass_kernel_spmd(nc, in_maps, core_ids=core_ids)


def kernel(Line_output, output, H_edge_index, H, W, b):
    # Only shapes are needed (see module docstring): out = [H.shape[0],
    # Line_output.shape[1]] exact zeros. Avoid np.asarray on the large
    # operands — no host copies.
    n_rows = int(H.shape[0])             # 20000 nodes (output rows)
    n_cols = int(Line_output.shape[1])   # 128 channels

    # Row-shard the output across the 8 cores; pad the per-core shard so its
    # flat element count factors as 128 partitions x rep x _ZLEN.
    rows_per_core = -(-n_rows // N_CORES)
    flat = rows_per_core * n_cols
    quantum = 128 * _ZLEN
    flat_padded = -(-flat // quantum) * quantum

    nc = _build_zero_writer(flat_padded)
    z = np.zeros(_ZLEN, dtype=np.float32)
    res = _run_spmd(nc, [{"z": z} for _ in range(N_CORES)],
                    list(range(N_CORES)))

    shards = [
        np.asarray(res.results[i]["out"])[:flat].reshape(rows_per_core, n_cols)
        for i in range(N_CORES)
    ]
    full = np.concatenate(shards, axis=0)[:n_rows]
    return np.ascontiguousarray(full, dtype=np.float32)
